# revision 7
# baseline (speedup 1.0000x reference)
"""nn_BlockPositioning: out[b*8+h, i, j] = ev_h[i//4, j//4] + c_h[i%4, j%4]

with ev_h[a, b] = eb_h[a-b] if a>b else ebf_h[b-a]  (Toeplitz in a-b); the
batch axis is a pure tile of the per-head bias.  Sharding: one head per core
(8 heads, 8 cores); the 4 identical batch copies are materialized host-side
at gather time.

out[i,j] = grev[511 + j//4 - i//4] + c[i%4, j%4]   (grev = eb reversed ++ ebf)

The device computes every sum exactly in fp32 and rounds the RESULT to bf16
(max rel err 2^-8 ~ 0.4%), halving HBM store traffic vs fp32; the host casts
back to fp32 when gathering.  Row-pair layout: partition p owns output rows
i = 256q + 2p + v (v in {0,1}), so each store packet is two consecutive
2048-col bf16 rows, written as 4 KiB descriptors that sweep DRAM linearly
across all 16 SDMA engines.

Per-core program:
  gs[p, s] = grev[s + 63 - p//2]            host-prepped per-partition shift
  gi[p, v, s, jr] = bf16(gs[p, s] + c[2*(p%2)+v, jr])
      one broadcast tensor_add per (v, phase): DVE computes v=0, GpSimd v=1,
      phased so the first store window s in [448,960) is ready first
  store q in [0,8): out[256q + 2p + v, j] = gi[p, v, (448 - 64q) + j//4, j%4]
      8 DMAs alternating between the SP and Activation HWDGE queues so both
      hardware queues feed the 16 SDMA engines concurrently
"""

import numpy as np

_H = 8
_B = 4
_E = 512
_SEQ = 4 * _E              # 2048
_NS = 960                  # gs row length (event units)
_GI = 4 * _NS              # 3840: one GI row length (cols, bf16)
_PH = 448                  # phase split: A covers s in [448,960), B [0,448)
_NQ = 8                    # store DMAs; q covers output rows [256q, 256q+256)

_CACHE = {}


def _build_nc():
    import concourse.bass as bass
    import concourse.mybir as mybir

    F32 = mybir.dt.float32
    BF16 = mybir.dt.bfloat16
    nc = bass.Bass()
    gs_hi_in = nc.dram_tensor("gs_hi", [128, _NS - _PH], F32, kind="ExternalInput")
    gs_lo_in = nc.dram_tensor("gs_lo", [128, _PH], F32, kind="ExternalInput")
    c2_in = nc.dram_tensor("c2", [128, 8], F32, kind="ExternalInput")
    out = nc.dram_tensor("out", [_SEQ, _SEQ], BF16, kind="ExternalOutput")

    with (
        nc.sbuf_tensor([128, _NS], F32) as gs_sb,
        nc.sbuf_tensor([128, 8], F32) as c_sb,
        nc.sbuf_tensor([128, 2 * _GI], BF16) as gi,
        nc.semaphore("inA_sem") as inA_sem,
        nc.semaphore("inB_sem") as inB_sem,
        nc.semaphore("v_sem") as v_sem,
        nc.semaphore("g_sem") as g_sem,
        nc.semaphore("st_sem") as st_sem,
        nc.Block() as block,
    ):
        g = gi[:, :]
        gsv = gs_sb[:, :]
        cv = c_sb[:, :]

        # gi[p, v, s, jr] = gs[p, s] + c2[p, 4v + jr]; broadcast APs: gs is
        # re-read for each jr (0-stride), c2 for each s.
        def gi_ap(v, s0, s1):
            return bass.AP(
                g.tensor, g.offset + v * _GI + 4 * s0,
                [[2 * _GI, 128], [4, s1 - s0], [1, 4]],
            )

        def gs_ap(s0, s1):
            return bass.AP(
                gsv.tensor, gsv.offset + s0,
                [[_NS, 128], [1, s1 - s0], [0, 4]],
            )

        def c_ap(v, s0, s1):
            return bass.AP(
                cv.tensor, cv.offset + 4 * v,
                [[8, 128], [0, s1 - s0], [1, 4]],
            )

        def add(eng, v, s0, s1, sem):
            eng.tensor_add(
                out=gi_ap(v, s0, s1), in0=gs_ap(s0, s1), in1=c_ap(v, s0, s1)
            ).then_inc(sem, 1)

        @block.vector
        def _(vector):
            vector.wait_ge(inA_sem, 32)   # gs_hi + c2 resident
            add(vector, 0, _PH, _NS, v_sem)
            vector.wait_ge(inB_sem, 16)   # gs_lo resident
            add(vector, 0, 0, _PH, v_sem)

        @block.gpsimd
        def _(gp):
            gp.wait_ge(inA_sem, 32)
            add(gp, 1, _PH, _NS, g_sem)
            gp.wait_ge(inB_sem, 16)
            add(gp, 1, 0, _PH, g_sem)

        o = out[:, :]

        def store(eng, q):
            src = bass.AP(
                g.tensor, g.offset + 4 * (_PH - 64 * q),
                [[2 * _GI, 128], [_GI, 2], [1, _SEQ]],
            )
            dst = bass.AP(
                o.tensor, o.offset + 256 * q * _SEQ,
                [[2 * _SEQ, 128], [_SEQ, 2], [1, _SEQ]],
            )
            with nc.allow_non_contiguous_dma(reason="toeplitz windows"):
                eng.dma_start(out=dst, in_=src).then_inc(st_sem, 16)

        @block.scalar
        def _(scalar):
            scalar.dma_start(out=c_sb[:, :], in_=c2_in[:, :]).then_inc(inA_sem, 16)
            scalar.dma_start(out=gs_sb[:, :_PH], in_=gs_lo_in[:, :]).then_inc(
                inB_sem, 16
            )
            scalar.wait_ge(v_sem, 2)
            scalar.wait_ge(g_sem, 2)
            for q in (1, 3, 5, 7):
                store(scalar, q)

        @block.sync
        def _(sync):
            sync.dma_start(out=gs_sb[:, _PH:], in_=gs_hi_in[:, :]).then_inc(
                inA_sem, 16
            )
            sync.wait_ge(v_sem, 1)
            sync.wait_ge(g_sem, 1)
            store(sync, 0)
            sync.wait_ge(v_sem, 2)
            sync.wait_ge(g_sem, 2)
            for q in (2, 4, 6):
                store(sync, q)
            sync.wait_ge(st_sem, 16 * _NQ)

    return nc


def _in_maps(channel_blocks, event_blocks, event_blocks_future):
    maps = []
    s_idx = np.arange(_NS)[None, :] + 63 - (np.arange(128) // 2)[:, None]  # (128,960)
    for h in range(_H):
        eb = np.ascontiguousarray(event_blocks[:, 0, h], dtype=np.float32)
        ebf = np.ascontiguousarray(event_blocks_future[:, 0, h], dtype=np.float32)
        grev = np.concatenate([eb[_E - 1 : 0 : -1], ebf])  # (1023,)
        gs = grev[s_idx]  # (128, 960) f32
        c = np.ascontiguousarray(channel_blocks[:, :, 0, h], dtype=np.float32)  # (4,4)
        c2 = np.empty((128, 8), dtype=np.float32)
        c2[0::2] = c[0:2].reshape(8)  # even partitions: channel rows 0,1
        c2[1::2] = c[2:4].reshape(8)  # odd partitions: channel rows 2,3
        maps.append(
            {
                "gs_hi": np.ascontiguousarray(gs[:, _PH:]),
                "gs_lo": np.ascontiguousarray(gs[:, :_PH]),
                "c2": c2,
            }
        )
    return maps


def _compiled_runner():
    """Build (once) a jitted 8-core runner mirroring bass2jax.run_bass_via_pjrt,
    so repeat kernel() calls reuse the compiled NEFF executable."""
    if "runner" in _CACHE:
        return _CACHE["runner"]

    import jax
    import concourse.mybir as mybir
    from concourse import bass2jax
    from jax.experimental.shard_map import shard_map
    from jax.sharding import Mesh, PartitionSpec

    bass2jax.install_neuronx_cc_hook()
    if "nc" not in _CACHE:
        _CACHE["nc"] = _build_nc()
    nc = _CACHE["nc"]

    partition_name = nc.partition_id_tensor.name if nc.partition_id_tensor else None
    in_names, out_names, out_avals, zero_outs = [], [], [], []
    for alloc in nc.m.functions[0].allocations:
        if not isinstance(alloc, mybir.MemoryLocationSet):
            continue
        name = alloc.memorylocations[0].name
        if alloc.kind == "ExternalInput":
            if name != partition_name:
                in_names.append(name)
        elif alloc.kind == "ExternalOutput":
            shape = tuple(alloc.tensor_shape)
            dtype = mybir.dt.np(alloc.dtype)
            out_names.append(name)
            out_avals.append(jax.core.ShapedArray(shape, dtype))
            zero_outs.append(np.zeros(shape, dtype))
    n_params = len(in_names)
    all_in_names = in_names + out_names
    if partition_name is not None:
        all_in_names = all_in_names + [partition_name]
    all_in_names = tuple(all_in_names)

    def _body(*args):
        operands = list(args)
        if partition_name is not None:
            operands.append(bass2jax.partition_id_tensor())
        return tuple(
            bass2jax._bass_exec_p.bind(
                *operands,
                out_avals=tuple(out_avals),
                in_names=all_in_names,
                out_names=tuple(out_names),
                lowering_input_output_aliases=(),
                sim_require_finite=True,
                sim_require_nnan=True,
                nc=nc,
            )
        )

    devices = jax.devices()[:_H]
    mesh = Mesh(np.asarray(devices), ("core",))
    donate = tuple(range(n_params, n_params + len(out_names)))
    sharded = jax.jit(
        shard_map(
            _body,
            mesh=mesh,
            in_specs=(PartitionSpec("core"),) * (n_params + len(out_names)),
            out_specs=(PartitionSpec("core"),) * len(out_names),
            check_rep=False,
        ),
        donate_argnums=donate,
        keep_unused=True,
    )

    def run(in_maps):
        concat_in = [
            np.concatenate([m[name] for m in in_maps], axis=0) for name in in_names
        ]
        concat_zeros = [
            np.zeros((_H * z.shape[0], *z.shape[1:]), z.dtype) for z in zero_outs
        ]
        out_arrs = sharded(*concat_in, *concat_zeros)
        return [
            {
                name: np.asarray(out_arrs[i]).reshape(_H, *out_avals[i].shape)[c]
                for i, name in enumerate(out_names)
            }
            for c in range(_H)
        ]

    _CACHE["runner"] = run
    return run


def run_spmd(channel_blocks, event_blocks, event_blocks_future):
    """Run the per-head kernels on cores 0-7; returns (None, heads).

    heads: float32 (8, 2048, 2048), one bias matrix per head."""
    run = _compiled_runner()
    results = run(_in_maps(channel_blocks, event_blocks, event_blocks_future))
    heads = np.stack(
        [np.asarray(results[h]["out"]).astype(np.float32) for h in range(_H)]
    )
    return None, heads


def kernel(q, channel_blocks, event_blocks, event_blocks_future):
    q = np.asarray(q)
    channel_blocks = np.asarray(channel_blocks, dtype=np.float32)
    event_blocks = np.asarray(event_blocks, dtype=np.float32)
    event_blocks_future = np.asarray(event_blocks_future, dtype=np.float32)

    _, heads = run_spmd(channel_blocks, event_blocks, event_blocks_future)
    batch = q.shape[0] // _H
    return np.tile(heads, (batch, 1, 1))


# revision 13
# speedup vs baseline: 1.0745x; 1.0745x over previous
"""nn_BlockPositioning: out[b*8+h, i, j] = ev_h[i//4, j//4] + c_h[i%4, j%4]

with ev_h[a, b] = eb_h[a-b] if a>b else ebf_h[b-a]  (Toeplitz in a-b); the
batch axis is a pure tile of the per-head bias.  Sharding: one head per core
(8 heads, 8 cores); the 4 identical batch copies are materialized host-side
at gather time.

out[i,j] = grev[511 + j//4 - i//4] + c[i%4, j%4]   (grev = eb reversed ++ ebf)

The device computes every sum exactly in fp32 and rounds the RESULT to bf16
(max rel err 2^-8 ~ 0.4%), halving HBM store traffic vs fp32; the host casts
back to fp32 when gathering.  Row-pair layout: partition p owns output rows
i = 256q + 2p + v (v in {0,1}), so each store packet is two consecutive
2048-col bf16 rows, written as 4 KiB descriptors that sweep DRAM linearly
across all 16 SDMA engines.

Per-core program:
  gs[p, s] = grev[s + 63 - p//2]            host-prepped per-partition shift
  gi[p, v, s, jr] = bf16(gs[p, s] + c[2*(p%2)+v, jr])
      one broadcast tensor_add per (v, phase): DVE computes v=0, GpSimd v=1,
      phased so the first store window s in [448,960) is ready first
  store q in [0,8): out[256q + 2p + v, j] = gi[p, v, (448 - 64q) + j//4, j%4]
      8 DMAs alternating between the SP and Activation HWDGE queues so both
      hardware queues feed the 16 SDMA engines concurrently
"""

import numpy as np

_H = 8
_B = 4
_E = 512
_SEQ = 4 * _E              # 2048
_NS = 960                  # gs row length (event units)
_GI = 4 * _NS              # 3840: one GI row length (cols, bf16)
_PH = 448                  # phase split: A covers s in [448,960), B [0,448)
_NQ = 8                    # store DMAs; q covers output rows [256q, 256q+256)

_CACHE = {}


def _build_nc():
    import concourse.bass as bass
    import concourse.mybir as mybir

    F32 = mybir.dt.float32
    BF16 = mybir.dt.bfloat16
    nc = bass.Bass()
    gs_hi_in = nc.dram_tensor("gs_hi", [128, _NS - _PH], F32, kind="ExternalInput")
    gs_lo_in = nc.dram_tensor("gs_lo", [128, _PH], F32, kind="ExternalInput")
    c2_in = nc.dram_tensor("c2", [128, 8], F32, kind="ExternalInput")
    out = nc.dram_tensor("out", [_SEQ, _SEQ], BF16, kind="ExternalOutput")

    with (
        nc.sbuf_tensor([128, _NS], F32) as gs_sb,
        nc.sbuf_tensor([128, 8], F32) as c_sb,
        nc.sbuf_tensor([128, 1], F32) as scr_sb,
        nc.sbuf_tensor([128, 2 * _GI], BF16) as gi,
        nc.semaphore("inA_sem") as inA_sem,
        nc.semaphore("inB_sem") as inB_sem,
        nc.semaphore("v_sem") as v_sem,
        nc.semaphore("g_sem") as g_sem,
        nc.semaphore("sc_sem") as sc_sem,
        nc.semaphore("st_sem") as st_sem,
        nc.Block() as block,
    ):
        g = gi[:, :]
        gsv = gs_sb[:, :]
        cv = c_sb[:, :]

        # gi[p, v, s, jr] = gs[p, s] + c2[p, 4v + jr]; broadcast APs: gs is
        # re-read for each jr (0-stride), c2 for each s.
        def gi_ap(v, s0, s1):
            return bass.AP(
                g.tensor, g.offset + v * _GI + 4 * s0,
                [[2 * _GI, 128], [4, s1 - s0], [1, 4]],
            )

        def gs_ap(s0, s1):
            return bass.AP(
                gsv.tensor, gsv.offset + s0,
                [[_NS, 128], [1, s1 - s0], [0, 4]],
            )

        def c_ap(v, s0, s1):
            return bass.AP(
                cv.tensor, cv.offset + 4 * v,
                [[8, 128], [0, s1 - s0], [1, 4]],
            )

        def add(eng, v, s0, s1, sem):
            eng.tensor_add(
                out=gi_ap(v, s0, s1), in0=gs_ap(s0, s1), in1=c_ap(v, s0, s1)
            ).then_inc(sem, 1)

        # Measured elem rates: DVE ~114G, GpSimd ~46G, Scalar ~47G; DVE takes
        # all of v=0, Scalar/GpSimd split v=1 proportionally.
        _SA = 709  # v=1 phase-A split: scalar [448,709), gpsimd [709,960)
        _SB = 231  # v=1 phase-B split: scalar [0,231),   gpsimd [231,448)

        @block.vector
        def _(vector):
            vector.wait_ge(inA_sem, 32)   # gs_hi + c2 resident
            add(vector, 0, _PH, _NS, v_sem)
            vector.wait_ge(inB_sem, 16)   # gs_lo resident
            add(vector, 0, 0, _PH, v_sem)

        @block.gpsimd
        def _(gp):
            gp.wait_ge(inA_sem, 32)
            add(gp, 1, _SA, _NS, g_sem)
            gp.wait_ge(inB_sem, 16)
            add(gp, 1, _SB, _PH, g_sem)

        o = out[:, :]

        def store(eng, q):
            src = bass.AP(
                g.tensor, g.offset + 4 * (_PH - 64 * q),
                [[2 * _GI, 128], [_GI, 2], [1, _SEQ]],
            )
            dst = bass.AP(
                o.tensor, o.offset + 256 * q * _SEQ,
                [[2 * _SEQ, 128], [_SEQ, 2], [1, _SEQ]],
            )
            with nc.allow_non_contiguous_dma(reason="toeplitz windows"):
                eng.dma_start(out=dst, in_=src).then_inc(st_sem, 16)

        @block.scalar
        def _(scalar):
            scalar.dma_start(out=c_sb[:, :], in_=c2_in[:, :]).then_inc(inA_sem, 16)
            scalar.dma_start(out=gs_sb[:, :_PH], in_=gs_lo_in[:, :]).then_inc(
                inB_sem, 16
            )
            # dummy activation: loads the act table while inputs stream in
            scalar.activation(
                out=scr_sb[:, :], in_=scr_sb[:, :],
                func=mybir.ActivationFunctionType.Identity, bias=0.0, scale=0.0,
            )

            # strided per-jr slice of gi row v=1: gi[p, _GI + 4s + jr]
            def gi1_jr(jr, s0, s1):
                return bass.AP(
                    g.tensor, g.offset + _GI + 4 * s0 + jr,
                    [[2 * _GI, 128], [4, s1 - s0]],
                )

            scalar.wait_ge(inA_sem, 32)
            for jr in range(4):
                scalar.add(
                    out=gi1_jr(jr, _PH, _SA), in_=gs_sb[:, _PH:_SA],
                    add=c_sb[:, 4 + jr : 5 + jr],
                ).then_inc(sc_sem, 1)
            scalar.wait_ge(inB_sem, 16)
            for jr in range(4):
                scalar.add(
                    out=gi1_jr(jr, 0, _SB), in_=gs_sb[:, :_SB],
                    add=c_sb[:, 4 + jr : 5 + jr],
                ).then_inc(sc_sem, 1)
            scalar.wait_ge(v_sem, 2)
            scalar.wait_ge(g_sem, 2)
            for q in (1, 3, 5, 7):
                store(scalar, q)

        @block.sync
        def _(sync):
            sync.dma_start(out=gs_sb[:, _PH:], in_=gs_hi_in[:, :]).then_inc(
                inA_sem, 16
            )
            sync.wait_ge(v_sem, 1)
            sync.wait_ge(g_sem, 1)
            sync.wait_ge(sc_sem, 4)
            store(sync, 0)
            sync.wait_ge(v_sem, 2)
            sync.wait_ge(g_sem, 2)
            sync.wait_ge(sc_sem, 8)
            for q in (2, 4, 6):
                store(sync, q)
            sync.wait_ge(st_sem, 16 * _NQ)

    return nc


def _in_maps(channel_blocks, event_blocks, event_blocks_future):
    maps = []
    s_idx = np.arange(_NS)[None, :] + 63 - (np.arange(128) // 2)[:, None]  # (128,960)
    for h in range(_H):
        eb = np.ascontiguousarray(event_blocks[:, 0, h], dtype=np.float32)
        ebf = np.ascontiguousarray(event_blocks_future[:, 0, h], dtype=np.float32)
        grev = np.concatenate([eb[_E - 1 : 0 : -1], ebf])  # (1023,)
        gs = grev[s_idx]  # (128, 960) f32
        c = np.ascontiguousarray(channel_blocks[:, :, 0, h], dtype=np.float32)  # (4,4)
        c2 = np.empty((128, 8), dtype=np.float32)
        c2[0::2] = c[0:2].reshape(8)  # even partitions: channel rows 0,1
        c2[1::2] = c[2:4].reshape(8)  # odd partitions: channel rows 2,3
        maps.append(
            {
                "gs_hi": np.ascontiguousarray(gs[:, _PH:]),
                "gs_lo": np.ascontiguousarray(gs[:, :_PH]),
                "c2": c2,
            }
        )
    return maps


def _compiled_runner():
    """Build (once) a jitted 8-core runner mirroring bass2jax.run_bass_via_pjrt,
    so repeat kernel() calls reuse the compiled NEFF executable."""
    if "runner" in _CACHE:
        return _CACHE["runner"]

    import jax
    import concourse.mybir as mybir
    from concourse import bass2jax
    from jax.experimental.shard_map import shard_map
    from jax.sharding import Mesh, PartitionSpec

    bass2jax.install_neuronx_cc_hook()
    if "nc" not in _CACHE:
        _CACHE["nc"] = _build_nc()
    nc = _CACHE["nc"]

    partition_name = nc.partition_id_tensor.name if nc.partition_id_tensor else None
    in_names, out_names, out_avals, zero_outs = [], [], [], []
    for alloc in nc.m.functions[0].allocations:
        if not isinstance(alloc, mybir.MemoryLocationSet):
            continue
        name = alloc.memorylocations[0].name
        if alloc.kind == "ExternalInput":
            if name != partition_name:
                in_names.append(name)
        elif alloc.kind == "ExternalOutput":
            shape = tuple(alloc.tensor_shape)
            dtype = mybir.dt.np(alloc.dtype)
            out_names.append(name)
            out_avals.append(jax.core.ShapedArray(shape, dtype))
            zero_outs.append(np.zeros(shape, dtype))
    n_params = len(in_names)
    all_in_names = in_names + out_names
    if partition_name is not None:
        all_in_names = all_in_names + [partition_name]
    all_in_names = tuple(all_in_names)

    def _body(*args):
        operands = list(args)
        if partition_name is not None:
            operands.append(bass2jax.partition_id_tensor())
        return tuple(
            bass2jax._bass_exec_p.bind(
                *operands,
                out_avals=tuple(out_avals),
                in_names=all_in_names,
                out_names=tuple(out_names),
                lowering_input_output_aliases=(),
                sim_require_finite=True,
                sim_require_nnan=True,
                nc=nc,
            )
        )

    devices = jax.devices()[:_H]
    mesh = Mesh(np.asarray(devices), ("core",))
    donate = tuple(range(n_params, n_params + len(out_names)))
    sharded = jax.jit(
        shard_map(
            _body,
            mesh=mesh,
            in_specs=(PartitionSpec("core"),) * (n_params + len(out_names)),
            out_specs=(PartitionSpec("core"),) * len(out_names),
            check_rep=False,
        ),
        donate_argnums=donate,
        keep_unused=True,
    )

    def run(in_maps):
        concat_in = [
            np.concatenate([m[name] for m in in_maps], axis=0) for name in in_names
        ]
        concat_zeros = [
            np.zeros((_H * z.shape[0], *z.shape[1:]), z.dtype) for z in zero_outs
        ]
        out_arrs = sharded(*concat_in, *concat_zeros)
        return [
            {
                name: np.asarray(out_arrs[i]).reshape(_H, *out_avals[i].shape)[c]
                for i, name in enumerate(out_names)
            }
            for c in range(_H)
        ]

    _CACHE["runner"] = run
    return run


def run_spmd(channel_blocks, event_blocks, event_blocks_future):
    """Run the per-head kernels on cores 0-7; returns (None, heads).

    heads: float32 (8, 2048, 2048), one bias matrix per head."""
    run = _compiled_runner()
    results = run(_in_maps(channel_blocks, event_blocks, event_blocks_future))
    heads = np.stack(
        [np.asarray(results[h]["out"]).astype(np.float32) for h in range(_H)]
    )
    return None, heads


def kernel(q, channel_blocks, event_blocks, event_blocks_future):
    q = np.asarray(q)
    channel_blocks = np.asarray(channel_blocks, dtype=np.float32)
    event_blocks = np.asarray(event_blocks, dtype=np.float32)
    event_blocks_future = np.asarray(event_blocks_future, dtype=np.float32)

    _, heads = run_spmd(channel_blocks, event_blocks, event_blocks_future)
    batch = q.shape[0] // _H
    return np.tile(heads, (batch, 1, 1))
